# revision 4
# baseline (speedup 1.0000x reference)
"""LIF spike kernel for Trainium2 (Bass/Tile), data-parallel over 8 NeuronCores.

Problem: x [32, 8, 128, 32, 32] fp32 -> spikes [32, 8, 128, 32, 32] fp32
    mem_t = mem_{t-1} * 0.25 + x_t ; spike = (mem >= 0.5) ; mem *= (1 - spike)

Sharding: batch dim (32) split 4-per-core across 8 cores; no cross-core comm.

Per-core device program, variant "packed" (default):
  - x host-sharded to c-major [4, 128, 8, 1024]; per batch ONE contiguous
    4 MiB dma_start into SBUF [128, 8*1024] (32 KiB per-partition runs).
  - per time step t on [128, 1024] slices:
      DVE : u_t = select(u_{t-1} < 0.5, TAU*u_{t-1}, 0) + x_t
            (one fused custom-DVE op; bitwise-exact vs the jax reference)
      Pool: s_t = (u_t >= 0.5) -> bf16 {0,1}    (tensor_scalar is_ge)
      PE  : pack 8 channels/byte: psum[16t+m, w] = sum_j 2^(7-j) s[8m+j, w]
            via matmul with a power-of-two weight matrix. Output rows of a
            16-row block must start at a multiple of 32, so even/odd t pairs
            share a 32-row block using zero-padded weights [W|0] / [0|W]
            and PSUM accumulate (start on even t, stop on odd t).
  - ACT : one PSUM->SBUF uint8 copy per batch ([128,1024], values 0..255
          exact in fp32) and one 128 KiB store per batch on the ACT ring.
  - HBM traffic per core: 16.78 MB read + 0.52 MB write (vs 4.2 MB write
    unpacked) -- the kernel is read-bandwidth-bound.
  - host decode: np.unpackbits(y, axis=2) gives [B, T, C, HW] {0,1} directly
    (bit j MSB-first of psum row 16t+m, column w == spike of channel 8m+j).
All spike-train arithmetic rounds identically to the jax reference, and the
pack path (bf16 {0,1} spikes, power-of-two bf16 weights, fp32 PSUM sums
<= 255) is exact, so the output matches bitwise.

Variant "fused" is the previous unpacked baseline (uint8 spikes stored
directly, 4 MiB store per core).
"""

import os
import numpy as np

B, T, C, H, W = 32, 8, 128, 32, 32
HW = H * W
N_CORES = 8
BPC = B // N_CORES  # batches per core
TAU = 0.25
THRESH = 0.5
CG = C // 8  # channel groups per byte column block (16)

_nc_cache = {}
LAST_RESULTS = None
_LIF_OP = None


def _register_lif_op():
    """Register the fused LIF-step custom DVE op with concourse's runtime
    table (the documented extension point is appending to dve_ops.OPS).

        out = select(in0 < s0, in0 * s1, 0) + in1
            = TAU*u*[u < THRESH] + x      (s0=THRESH, s1=TAU)

    One DVE instruction per time step instead of two scalar_tensor_tensor
    passes; exact fp32 (mult by 2^-2 exact, one rounding add)."""
    global _LIF_OP
    if _LIF_OP is not None:
        return _LIF_OP
    from concourse import dve_ops
    from concourse.dve_spec import (
        Spec,
        Src0,
        Src1,
        C0,
        C1,
        Zero,
        select,
        lower,
        _has_src1,
    )
    from concourse.dve_uop import DveOpSpec

    name = "LIF_STEP_ANT"
    for op in dve_ops.OPS:
        if op.name == name:
            _LIF_OP = op
            return op

    body = select(Src0 < C0, Src0 * C1, Zero) + Src1

    def _ref(in0, in1, s0, s1, imm2):
        return (
            np.where(in0 < s0, in0.astype(np.float32) * s1, 0.0).astype(np.float32)
            + in1
        )

    spec = Spec(body=body, reference=_ref)
    opcode = dve_ops._CUSTOM_DVE_ROW_BASE + len(dve_ops.OPS)
    assert opcode < 0x20
    shas = {}
    for ver in ("v3", "v4"):
        try:
            uops = lower(spec, ver=ver)
        except Exception:
            continue
        shas[ver] = DveOpSpec(
            name=name, opcode=opcode, uops=uops, rd1_en=_has_src1(spec)
        ).sha(ver)
    op = dve_ops.DveOp(name, spec, subdim=False, uops_sha=shas)
    dve_ops.OPS.append(op)
    dve_ops._SUB_OPCODE_FOR_NAME[name] = opcode
    dve_ops.CUSTOM_DVE_SPECS[name] = spec
    _LIF_OP = op
    return op


def pack_weights():
    """W [128, 16] with W[8m+j, m] = 2^(7-j): matmul W.T @ s packs the 8
    spike channels of group m into one byte value, MSB = lowest channel,
    matching np.unpackbits order."""
    w = np.zeros((C, CG), np.float32)
    for m in range(CG):
        for j in range(8):
            w[8 * m + j, m] = float(2 ** (7 - j))
    return w


def build_bass(reps=1, variant="packed"):
    """Per-core Bass program. reps>1 repeats the whole computation for
    loop-delta hardware timing (outputs are rewritten identically)."""
    import concourse.bacc as bacc
    import concourse.mybir as mybir
    from concourse.tile import TileContext

    f32 = mybir.dt.float32
    bf16 = mybir.dt.bfloat16
    u8 = mybir.dt.uint8
    Alu = mybir.AluOpType
    AF = mybir.ActivationFunctionType
    lif_op = _register_lif_op()

    nc = bacc.Bacc("TRN2", target_bir_lowering=False)
    x = nc.dram_tensor("x", [BPC, C, T, HW], f32, kind="ExternalInput")

    if variant == "fused":
        y = nc.dram_tensor("y", [BPC, C, T, HW], u8, kind="ExternalOutput")
        with TileContext(nc) as tc:
            with (
                tc.tile_pool(name="xp", bufs=3) as xp,
                tc.tile_pool(name="up", bufs=3) as up,
                tc.tile_pool(name="yp", bufs=2) as yp,
                tc.tile_pool(name="cp", bufs=1) as cp,
            ):
                neg_thresh = cp.tile([C, 1], f32)
                nc.vector.memset(neg_thresh[:], -THRESH)
                for _rep in range(reps):
                    for b in range(BPC):
                        xb = xp.tile([C, T, HW], f32, tag="xb")
                        nc.sync.dma_start(xb[:], x[b])
                        if b == 0:
                            yg = yp.tile([C, BPC, T, HW], u8, tag="yb")
                        yb = yg[:, b]
                        u = None
                        for t in range(T):
                            xt = xb[:, t, :]
                            if t == 0:
                                u = xt
                            else:
                                un = up.tile([C, HW], f32, tag="u")
                                nc.vector._custom_dve(
                                    lif_op,
                                    out=un[:],
                                    in0=u if t == 1 else u[:],
                                    in1=xt,
                                    s0=THRESH,
                                    s1=TAU,
                                )
                            u = u if t == 0 else un
                            uin = u if t == 0 else u[:]
                            nc.scalar.activation(
                                yb[:, t, :],
                                uin,
                                AF.Sign,
                                bias=neg_thresh[:],
                            )
                        if b == BPC - 1:
                            nc.scalar.dma_start(
                                y[:].rearrange("b c t w -> c b t w"), yg[:]
                            )
        nc.compile()
        return nc

    assert variant == "packed"
    w = nc.dram_tensor("w", [C, CG], bf16, kind="ExternalInput")
    y = nc.dram_tensor("y", [BPC, C, HW], u8, kind="ExternalOutput")

    with TileContext(nc) as tc:
        with (
            tc.tile_pool(name="xp", bufs=3) as xp,
            tc.tile_pool(name="up", bufs=3) as up,
            tc.tile_pool(name="sp", bufs=3) as spool,
            tc.tile_pool(name="yp", bufs=2) as yp,
            tc.tile_pool(name="pp", bufs=2, space="PSUM") as pp,
            tc.tile_pool(name="cp", bufs=1) as cp,
        ):
            # wsb[:, 0:16] = W, [:, 16:32] = 0, [:, 32:48] = W so that
            # [W|0] = wsb[:, 0:32] (even t) and [0|W] = wsb[:, 16:48] (odd t)
            wsb = cp.tile([C, 3 * CG], bf16)
            nc.gpsimd.memset(wsb[:], 0)
            nc.sync.dma_start(wsb[:, 0:CG], w[:])
            nc.sync.dma_start(wsb[:, 2 * CG : 3 * CG], w[:])
            for _rep in range(reps):
                for b in range(BPC):
                    xb = xp.tile([C, T, HW], f32, tag="xb")
                    # chunked loads: 1 MiB jobs (8 KiB descriptors) sustain
                    # ~1.1 TB/s vs ~0.6 TB/s for one 4 MiB job (measured)
                    for h in range(0, T, 2):
                        nc.sync.dma_start(xb[:, h : h + 2], x[b, :, h : h + 2])
                    ps = pp.tile([C, HW], f32, tag="ps")
                    u = None
                    for t in range(T):
                        xt = xb[:, t, :]
                        if t == 0:
                            u = xt
                        else:
                            un = up.tile([C, HW], f32, tag="u")
                            nc.vector._custom_dve(
                                lif_op,
                                out=un[:],
                                in0=u if t == 1 else u[:],
                                in1=xt,
                                s0=THRESH,
                                s1=TAU,
                            )
                            u = un
                        uin = u if t == 0 else u[:]
                        s = spool.tile([C, HW], bf16, tag="s")
                        nc.gpsimd.tensor_scalar(s[:], uin, THRESH, None, Alu.is_ge)
                        lhsT = (
                            wsb[:, 0 : 2 * CG]
                            if t % 2 == 0
                            else wsb[:, CG : 3 * CG]
                        )
                        r0 = 32 * (t // 2)
                        for h in range(2):
                            nc.tensor.matmul(
                                ps[r0 : r0 + 32, 512 * h : 512 * (h + 1)],
                                lhsT,
                                s[:, 512 * h : 512 * (h + 1)],
                                start=(t % 2 == 0),
                                stop=(t % 2 == 1),
                                # explicit: out.base_partition() rejects 96
                                tile_position=(0, r0),
                            )
                    ysb = yp.tile([C, HW], u8, tag="ysb")
                    nc.scalar.activation(ysb[:], ps[:], AF.Copy)
                    nc.scalar.dma_start(y[b], ysb[:])
    nc.compile()
    return nc


def _get_nc():
    variant = os.environ.get("LIF_VARIANT", "packed")
    if variant not in _nc_cache:
        _nc_cache[variant] = build_bass(variant=variant)
    return _nc_cache[variant], variant


def kernel(x):
    global LAST_RESULTS
    from concourse import bass_utils
    import ml_dtypes

    x = np.asarray(x)
    assert x.shape == (B, T, C, H, W) and x.dtype == np.float32
    # shard to per-core c-major [BPC, C, T, HW] (the copy happens anyway)
    xs = np.moveaxis(x.reshape(B, T, C, HW), 1, 2)
    nc, variant = _get_nc()
    if variant == "packed":
        wb = pack_weights().astype(ml_dtypes.bfloat16)
        in_maps = [
            {"x": np.ascontiguousarray(xs[i * BPC : (i + 1) * BPC]), "w": wb}
            for i in range(N_CORES)
        ]
    else:
        in_maps = [
            {"x": np.ascontiguousarray(xs[i * BPC : (i + 1) * BPC])}
            for i in range(N_CORES)
        ]
    res = bass_utils.run_bass_kernel_spmd(
        nc,
        in_maps,
        core_ids=list(range(N_CORES)),
        trace=bool(int(os.environ.get("LIF_TRACE", "0"))),
    )
    LAST_RESULTS = res
    out = np.empty((B, T, C, HW), dtype=np.float32)
    for i in range(N_CORES):
        yi = res.results[i]["y"]
        if variant == "packed":
            # yi [BPC, 128, 1024]: row 16t+m packs channels 8m..8m+7 MSB-first
            bits = np.unpackbits(yi.reshape(BPC, T, CG, HW), axis=2)
            out[i * BPC : (i + 1) * BPC] = bits  # [BPC, T, C, HW]
        else:
            # ACT Sign gives {-1,0,+1}; in uint8 that is {255,0,1}: spike == 1.
            out[i * BPC : (i + 1) * BPC] = np.moveaxis(yi, 1, 2) == 1
    return out.reshape(B, T, C, H, W)


# revision 7
# speedup vs baseline: 10.6648x; 10.6648x over previous
"""LIF spike kernel for Trainium2 (Bass/Tile), data-parallel over 8 NeuronCores.

Problem: x [32, 8, 128, 32, 32] fp32 -> spikes [32, 8, 128, 32, 32] fp32
    mem_t = mem_{t-1} * 0.25 + x_t ; spike = (mem >= 0.5) ; mem *= (1 - spike)

Sharding: batch dim (32) split 4-per-core across 8 cores; no cross-core comm.

Per-core device program, variant "direct" (default), measured-driven design:
  - loads: x host-sharded to c-major [4, 128, 8, 1024]; per batch FOUR 1 MiB
    dma_start jobs ([128, 2, 1024] slices, 8 KiB per-partition descriptors).
    Measured: chunked jobs sustain ~1.1 TB/s vs ~0.6 TB/s for one 4 MiB job,
    so the 16.8 MB read floor is ~15.5 us.
  - DVE: the serial LIF recurrence, one fused custom-DVE op per step
        u_t = select(u_{t-1} < 0.5, TAU*u_{t-1}, 0) + x_t
    (bitwise-exact vs the jax reference) plus the first LIF_G8 spike steps
    per batch (tensor_scalar is_ge -> uint8 {0,1}).
  - ACT: remaining spike steps via Sign(u - 0.5) -> uint8 {255,0,1}, two
    steps fused per instruction where possible (the paired membrane tiles
    are column-adjacent so one Sign covers [128, 2048]).
  - spikes accumulate per batch in a [128, 8*1024] uint8 tile; ONE 1 MiB
    store per batch issued from the otherwise-idle Pool (SWDGE) ring so
    store jobs never queue behind load jobs on SP.
  - host maps uint8 (y == 1) -> fp32: is_ge gives {0,1}, Sign gives
    {255,0,1}; spike == 1 under both, so no correction pass is needed.
  - measured engine costs: DVE LIF op ~550 ns, ACT Sign ~0.8/1.5 us
    (1024/2048 wide); both engines land ~18-20 us busy, just above the
    ~15.5 us DMA floor.
All arithmetic is fp32 and rounds identically to the jax reference, so the
spike train matches bitwise.

Variant "fused" is the previous baseline (all spikes on ACT, one 4 MiB
store at the end, unchunked loads).
"""

import os
import numpy as np

B, T, C, H, W = 32, 8, 128, 32, 32
HW = H * W
N_CORES = 8
BPC = B // N_CORES  # batches per core
TAU = 0.25
THRESH = 0.5

_nc_cache = {}
LAST_RESULTS = None
_LIF_OP = None


def _register_lif_op():
    """Register the fused LIF-step custom DVE op with concourse's runtime
    table (the documented extension point is appending to dve_ops.OPS).

        out = select(in0 < s0, in0 * s1, 0) + in1
            = TAU*u*[u < THRESH] + x      (s0=THRESH, s1=TAU)

    One DVE instruction per time step instead of two scalar_tensor_tensor
    passes; exact fp32 (mult by 2^-2 exact, one rounding add)."""
    global _LIF_OP
    if _LIF_OP is not None:
        return _LIF_OP
    from concourse import dve_ops
    from concourse.dve_spec import (
        Spec,
        Src0,
        Src1,
        C0,
        C1,
        Zero,
        select,
        lower,
        _has_src1,
    )
    from concourse.dve_uop import DveOpSpec

    name = "LIF_STEP_ANT"
    for op in dve_ops.OPS:
        if op.name == name:
            _LIF_OP = op
            return op

    body = select(Src0 < C0, Src0 * C1, Zero) + Src1

    def _ref(in0, in1, s0, s1, imm2):
        return (
            np.where(in0 < s0, in0.astype(np.float32) * s1, 0.0).astype(np.float32)
            + in1
        )

    spec = Spec(body=body, reference=_ref)
    opcode = dve_ops._CUSTOM_DVE_ROW_BASE + len(dve_ops.OPS)
    assert opcode < 0x20
    shas = {}
    for ver in ("v3", "v4"):
        try:
            uops = lower(spec, ver=ver)
        except Exception:
            continue
        shas[ver] = DveOpSpec(
            name=name, opcode=opcode, uops=uops, rd1_en=_has_src1(spec)
        ).sha(ver)
    op = dve_ops.DveOp(name, spec, subdim=False, uops_sha=shas)
    dve_ops.OPS.append(op)
    dve_ops._SUB_OPCODE_FOR_NAME[name] = opcode
    dve_ops.CUSTOM_DVE_SPECS[name] = spec
    _LIF_OP = op
    return op


def build_bass(reps=1, variant="direct"):
    """Per-core Bass program. reps>1 repeats the whole computation for
    loop-delta hardware timing (outputs are rewritten identically)."""
    import concourse.bacc as bacc
    import concourse.mybir as mybir
    from concourse.tile import TileContext

    f32 = mybir.dt.float32
    u8 = mybir.dt.uint8
    Alu = mybir.AluOpType
    AF = mybir.ActivationFunctionType
    lif_op = _register_lif_op()

    nc = bacc.Bacc("TRN2", target_bir_lowering=False)
    x = nc.dram_tensor("x", [BPC, C, T, HW], f32, kind="ExternalInput")
    y = nc.dram_tensor("y", [BPC, C, T, HW], u8, kind="ExternalOutput")

    if variant == "fused":
        with TileContext(nc) as tc:
            with (
                tc.tile_pool(name="xp", bufs=3) as xp,
                tc.tile_pool(name="up", bufs=3) as up,
                tc.tile_pool(name="yp", bufs=2) as yp,
                tc.tile_pool(name="cp", bufs=1) as cp,
            ):
                neg_thresh = cp.tile([C, 1], f32)
                nc.vector.memset(neg_thresh[:], -THRESH)
                for _rep in range(reps):
                    for b in range(BPC):
                        xb = xp.tile([C, T, HW], f32, tag="xb")
                        nc.sync.dma_start(xb[:], x[b])
                        if b == 0:
                            yg = yp.tile([C, BPC, T, HW], u8, tag="yb")
                        yb = yg[:, b]
                        u = None
                        for t in range(T):
                            xt = xb[:, t, :]
                            if t == 0:
                                u = xt
                            else:
                                un = up.tile([C, HW], f32, tag="u")
                                nc.vector._custom_dve(
                                    lif_op,
                                    out=un[:],
                                    in0=u if t == 1 else u[:],
                                    in1=xt,
                                    s0=THRESH,
                                    s1=TAU,
                                )
                                u = un
                            uin = u if t == 0 else u[:]
                            nc.scalar.activation(
                                yb[:, t, :], uin, AF.Sign, bias=neg_thresh[:]
                            )
                        if b == BPC - 1:
                            nc.scalar.dma_start(
                                y[:].rearrange("b c t w -> c b t w"), yg[:]
                            )
        nc.compile()
        return nc

    assert variant == "direct"
    g8 = int(os.environ.get("LIF_G8", "2"))  # spike steps on DVE per batch
    # ACT handles steps g8..7, two per Sign instruction where possible
    act_steps = list(range(g8, T))
    pairs = []
    i = 0
    while i < len(act_steps):
        n = 2 if i + 1 < len(act_steps) else 1
        pairs.append(tuple(act_steps[i : i + n]))
        i += n
    pair_of = {t: p for p in pairs for t in p}

    with TileContext(nc) as tc:
        with (
            tc.tile_pool(name="xp", bufs=3) as xp,
            tc.tile_pool(name="up", bufs=3) as up,
            tc.tile_pool(name="yp", bufs=2) as yp,
            tc.tile_pool(name="cp", bufs=1) as cp,
        ):
            neg_thresh = cp.tile([C, 1], f32)
            nc.vector.memset(neg_thresh[:], -THRESH)
            for _rep in range(reps):
                for b in range(BPC):
                    xb = xp.tile([C, T, HW], f32, tag="xb")
                    for h in range(0, T, 2):
                        nc.sync.dma_start(xb[:, h : h + 2], x[b, :, h : h + 2])
                    sy = yp.tile([C, T, HW], u8, tag="sy")
                    uap = {0: xb[:, 0, :]}
                    ptile = {}
                    for t in range(T):
                        if t >= 1:
                            p = pair_of.get(t)
                            if p is not None and len(p) == 2:
                                if t == p[0]:
                                    ptile[p] = up.tile([C, 2, HW], f32, tag="u2", name="u2")
                                dst = ptile[p][:, t - p[0], :]
                            else:
                                u1 = up.tile([C, HW], f32, tag="u1", name="u1")
                                dst = u1[:]
                            nc.vector._custom_dve(
                                lif_op,
                                out=dst,
                                in0=uap[t - 1],
                                in1=xb[:, t, :],
                                s0=THRESH,
                                s1=TAU,
                            )
                            uap[t] = dst
                        if t < g8:
                            # DVE spike: {0,1} uint8
                            nc.vector.tensor_scalar(
                                sy[:, t, :], uap[t], THRESH, None, Alu.is_ge
                            )
                        else:
                            p = pair_of[t]
                            if t == p[-1]:
                                # ACT spike(s): Sign -> {255,0,1} uint8
                                if len(p) == 2:
                                    nc.scalar.activation(
                                        sy[:, p[0] : p[0] + 2, :],
                                        ptile[p][:],
                                        AF.Sign,
                                        bias=neg_thresh[:],
                                    )
                                else:
                                    nc.scalar.activation(
                                        sy[:, t, :],
                                        uap[t],
                                        AF.Sign,
                                        bias=neg_thresh[:],
                                    )
                    # store rides the idle Pool SWDGE ring: never queues
                    # behind load jobs (SP) or spikes (ACT)
                    nc.gpsimd.dma_start(y[b], sy[:])
    nc.compile()
    return nc


def _get_nc():
    variant = os.environ.get("LIF_VARIANT", "direct")
    key = (variant, os.environ.get("LIF_G8", "2"))
    if key not in _nc_cache:
        _nc_cache[key] = build_bass(variant=variant)
    return _nc_cache[key], variant


def kernel(x):
    global LAST_RESULTS
    from concourse import bass_utils

    x = np.asarray(x)
    assert x.shape == (B, T, C, H, W) and x.dtype == np.float32
    # shard to per-core c-major [BPC, C, T, HW] (the copy happens anyway)
    xs = np.moveaxis(x.reshape(B, T, C, HW), 1, 2)
    nc, variant = _get_nc()
    in_maps = [
        {"x": np.ascontiguousarray(xs[i * BPC : (i + 1) * BPC])}
        for i in range(N_CORES)
    ]
    res = bass_utils.run_bass_kernel_spmd(
        nc,
        in_maps,
        core_ids=list(range(N_CORES)),
        trace=bool(int(os.environ.get("LIF_TRACE", "0"))),
    )
    LAST_RESULTS = res
    out = np.empty((B, T, C, HW), dtype=np.float32)
    for i in range(N_CORES):
        # DVE is_ge gives {0,1}; ACT Sign gives {-1,0,+1} = {255,0,1} in
        # uint8. spike == 1 under both conventions.
        yi = np.moveaxis(res.results[i]["y"], 1, 2)
        out[i * BPC : (i + 1) * BPC] = yi == 1
    return out.reshape(B, T, C, H, W)


# revision 10
# speedup vs baseline: 13.0066x; 1.2196x over previous
"""LIF spike kernel for Trainium2 (Bass/Tile), data-parallel over 8 NeuronCores.

Problem: x [32, 8, 128, 32, 32] fp32 -> spikes [32, 8, 128, 32, 32] fp32
    mem_t = mem_{t-1} * 0.25 + x_t ; spike = (mem >= 0.5) ; mem *= (1 - spike)

Sharding: batch dim (32) split 4-per-core across 8 cores; no cross-core comm.

Per-core device program, variant "direct" (default), measured-driven design:
  - loads: x host-sharded to c-major [4, 128, 8, 1024]; per batch FOUR 1 MiB
    dma_start jobs ([128, 2, 1024] slices, 8 KiB per-partition descriptors).
    Measured: chunked jobs sustain ~1.1 TB/s vs ~0.6 TB/s for one 4 MiB job,
    so the 16.8 MB read floor is ~15.5 us.
  - DVE: the serial LIF recurrence, one fused custom-DVE op per step
        u_t = select(u_{t-1} < 0.5, TAU*u_{t-1}, 0) + x_t
    (bitwise-exact vs the jax reference) plus the first LIF_G8 spike steps
    per batch (tensor_scalar is_ge -> uint8 {0,1}).
  - ACT: remaining spike steps via Sign(u - 0.5) -> uint8 {255,0,1}, two
    steps fused per instruction where possible (the paired membrane tiles
    are column-adjacent so one Sign covers [128, 2048]).
  - spikes accumulate per batch in a [128, 8*1024] uint8 tile; ONE 1 MiB
    store per batch issued from the otherwise-idle Pool (SWDGE) ring so
    store jobs never queue behind load jobs on SP.
  - host maps uint8 (y == 1) -> fp32: is_ge gives {0,1}, Sign gives
    {255,0,1}; spike == 1 under both, so no correction pass is needed.
  - measured engine costs: DVE LIF op ~550 ns, ACT Sign ~0.8/1.5 us
    (1024/2048 wide); both engines land ~18-20 us busy, just above the
    ~15.5 us DMA floor.
All arithmetic is fp32 and rounds identically to the jax reference, so the
spike train matches bitwise.

Variant "fused" is the previous baseline (all spikes on ACT, one 4 MiB
store at the end, unchunked loads).
"""

import os
import numpy as np

B, T, C, H, W = 32, 8, 128, 32, 32
HW = H * W
N_CORES = 8
BPC = B // N_CORES  # batches per core
TAU = 0.25
THRESH = 0.5

_nc_cache = {}
LAST_RESULTS = None
_LIF_OP = None


def _register_lif_op():
    """Register the fused LIF-step custom DVE op with concourse's runtime
    table (the documented extension point is appending to dve_ops.OPS).

        out = select(in0 < s0, in0 * s1, 0) + in1
            = TAU*u*[u < THRESH] + x      (s0=THRESH, s1=TAU)

    One DVE instruction per time step instead of two scalar_tensor_tensor
    passes; exact fp32 (mult by 2^-2 exact, one rounding add)."""
    global _LIF_OP
    if _LIF_OP is not None:
        return _LIF_OP
    from concourse import dve_ops
    from concourse.dve_spec import (
        Spec,
        Src0,
        Src1,
        C0,
        C1,
        Zero,
        select,
        lower,
        _has_src1,
    )
    from concourse.dve_uop import DveOpSpec

    name = "LIF_STEP_ANT"
    for op in dve_ops.OPS:
        if op.name == name:
            _LIF_OP = op
            return op

    body = select(Src0 < C0, Src0 * C1, Zero) + Src1

    def _ref(in0, in1, s0, s1, imm2):
        return (
            np.where(in0 < s0, in0.astype(np.float32) * s1, 0.0).astype(np.float32)
            + in1
        )

    spec = Spec(body=body, reference=_ref)
    opcode = dve_ops._CUSTOM_DVE_ROW_BASE + len(dve_ops.OPS)
    assert opcode < 0x20
    shas = {}
    for ver in ("v3", "v4"):
        try:
            uops = lower(spec, ver=ver)
        except Exception:
            continue
        shas[ver] = DveOpSpec(
            name=name, opcode=opcode, uops=uops, rd1_en=_has_src1(spec)
        ).sha(ver)
    op = dve_ops.DveOp(name, spec, subdim=False, uops_sha=shas)
    dve_ops.OPS.append(op)
    dve_ops._SUB_OPCODE_FOR_NAME[name] = opcode
    dve_ops.CUSTOM_DVE_SPECS[name] = spec
    _LIF_OP = op
    return op


def build_bass(reps=1, variant="direct"):
    """Per-core Bass program. reps>1 repeats the whole computation for
    loop-delta hardware timing (outputs are rewritten identically)."""
    import concourse.bacc as bacc
    import concourse.mybir as mybir
    from concourse.tile import TileContext

    f32 = mybir.dt.float32
    u8 = mybir.dt.uint8
    Alu = mybir.AluOpType
    AF = mybir.ActivationFunctionType
    lif_op = _register_lif_op()

    nc = bacc.Bacc("TRN2", target_bir_lowering=False)
    x = nc.dram_tensor("x", [BPC, C, T, HW], f32, kind="ExternalInput")
    y = nc.dram_tensor("y", [BPC, C, T, HW], u8, kind="ExternalOutput")

    if variant == "fused":
        with TileContext(nc) as tc:
            with (
                tc.tile_pool(name="xp", bufs=3) as xp,
                tc.tile_pool(name="up", bufs=3) as up,
                tc.tile_pool(name="yp", bufs=2) as yp,
                tc.tile_pool(name="cp", bufs=1) as cp,
            ):
                neg_thresh = cp.tile([C, 1], f32)
                nc.vector.memset(neg_thresh[:], -THRESH)
                for _rep in range(reps):
                    for b in range(BPC):
                        xb = xp.tile([C, T, HW], f32, tag="xb")
                        nc.sync.dma_start(xb[:], x[b])
                        if b == 0:
                            yg = yp.tile([C, BPC, T, HW], u8, tag="yb")
                        yb = yg[:, b]
                        u = None
                        for t in range(T):
                            xt = xb[:, t, :]
                            if t == 0:
                                u = xt
                            else:
                                un = up.tile([C, HW], f32, tag="u")
                                nc.vector._custom_dve(
                                    lif_op,
                                    out=un[:],
                                    in0=u if t == 1 else u[:],
                                    in1=xt,
                                    s0=THRESH,
                                    s1=TAU,
                                )
                                u = un
                            uin = u if t == 0 else u[:]
                            nc.scalar.activation(
                                yb[:, t, :], uin, AF.Sign, bias=neg_thresh[:]
                            )
                        if b == BPC - 1:
                            nc.scalar.dma_start(
                                y[:].rearrange("b c t w -> c b t w"), yg[:]
                            )
        nc.compile()
        return nc

    assert variant == "direct"
    g8 = int(os.environ.get("LIF_G8", "2"))  # spike steps on DVE per batch
    # ACT handles steps g8..7, two per Sign instruction where possible
    act_steps = list(range(g8, T))
    pairs = []
    i = 0
    while i < len(act_steps):
        n = 2 if i + 1 < len(act_steps) else 1
        pairs.append(tuple(act_steps[i : i + n]))
        i += n
    pair_of = {t: p for p in pairs for t in p}

    with TileContext(nc) as tc:
        with (
            tc.tile_pool(name="xp", bufs=3) as xp,
            tc.tile_pool(name="up", bufs=3) as up,
            tc.tile_pool(name="yp", bufs=2) as yp,
            tc.tile_pool(name="cp", bufs=1) as cp,
        ):
            neg_thresh = cp.tile([C, 1], f32)
            nc.vector.memset(neg_thresh[:], -THRESH)
            for _rep in range(reps):
                for b in range(BPC):
                    xb = xp.tile([C, T, HW], f32, tag="xb")
                    for h in range(0, T, 2):
                        nc.sync.dma_start(xb[:, h : h + 2], x[b, :, h : h + 2])
                    sy = yp.tile([C, T, HW], u8, tag="sy")
                    uap = {0: xb[:, 0, :]}
                    ptile = {}
                    for t in range(T):
                        if t >= 1:
                            p = pair_of.get(t)
                            if p is not None and len(p) == 2:
                                if t == p[0]:
                                    ptile[p] = up.tile([C, 2, HW], f32, tag="u2", name="u2")
                                dst = ptile[p][:, t - p[0], :]
                            else:
                                u1 = up.tile([C, HW], f32, tag="u1", name="u1")
                                dst = u1[:]
                            nc.vector._custom_dve(
                                lif_op,
                                out=dst,
                                in0=uap[t - 1],
                                in1=xb[:, t, :],
                                s0=THRESH,
                                s1=TAU,
                            )
                            uap[t] = dst
                        if t < g8:
                            # DVE spike: {0,1} uint8
                            nc.vector.tensor_scalar(
                                sy[:, t, :], uap[t], THRESH, None, Alu.is_ge
                            )
                        else:
                            p = pair_of[t]
                            if t == p[-1]:
                                # ACT spike(s): Sign -> {255,0,1} uint8
                                if len(p) == 2:
                                    nc.scalar.activation(
                                        sy[:, p[0] : p[0] + 2, :],
                                        ptile[p][:],
                                        AF.Sign,
                                        bias=neg_thresh[:],
                                    )
                                else:
                                    nc.scalar.activation(
                                        sy[:, t, :],
                                        uap[t],
                                        AF.Sign,
                                        bias=neg_thresh[:],
                                    )
                    # store off the SP ring so store jobs never queue
                    # behind load jobs
                    store_eng = {
                        "pool": nc.gpsimd,
                        "act": nc.scalar,
                        "sp": nc.sync,
                        "dve": nc.vector,
                    }[os.environ.get("LIF_STORE", "act")]
                    if os.environ.get("LIF_NOSTORE", "0") == "1":
                        # diagnostic: only the last rep stores (timing-only)
                        if _rep == reps - 1:
                            store_eng.dma_start(y[b], sy[:])
                    else:
                        store_eng.dma_start(y[b], sy[:])
    nc.compile()
    return nc


def _get_nc():
    variant = os.environ.get("LIF_VARIANT", "direct")
    key = (variant, os.environ.get("LIF_G8", "2"), os.environ.get("LIF_STORE", "act"))
    if key not in _nc_cache:
        _nc_cache[key] = build_bass(variant=variant)
    return _nc_cache[key], variant


def kernel(x):
    global LAST_RESULTS
    from concourse import bass_utils

    x = np.asarray(x)
    assert x.shape == (B, T, C, H, W) and x.dtype == np.float32
    # shard to per-core c-major [BPC, C, T, HW] (the copy happens anyway)
    xs = np.moveaxis(x.reshape(B, T, C, HW), 1, 2)
    nc, variant = _get_nc()
    in_maps = [
        {"x": np.ascontiguousarray(xs[i * BPC : (i + 1) * BPC])}
        for i in range(N_CORES)
    ]
    res = bass_utils.run_bass_kernel_spmd(
        nc,
        in_maps,
        core_ids=list(range(N_CORES)),
        trace=bool(int(os.environ.get("LIF_TRACE", "0"))),
    )
    LAST_RESULTS = res
    out = np.empty((B, T, C, HW), dtype=np.float32)
    for i in range(N_CORES):
        # DVE is_ge gives {0,1}; ACT Sign gives {-1,0,+1} = {255,0,1} in
        # uint8. spike == 1 under both conventions.
        yi = np.moveaxis(res.results[i]["y"], 1, 2)
        out[i * BPC : (i + 1) * BPC] = yi == 1
    return out.reshape(B, T, C, H, W)


# revision 11
# speedup vs baseline: 18.0601x; 1.3885x over previous
"""LIF spike kernel for Trainium2 (Bass/Tile), data-parallel over 8 NeuronCores.

Problem: x [32, 8, 128, 32, 32] fp32 -> spikes [32, 8, 128, 32, 32] fp32
    mem_t = mem_{t-1} * 0.25 + x_t ; spike = (mem >= 0.5) ; mem *= (1 - spike)

Sharding: batch dim (32) split 4-per-core across 8 cores; no cross-core comm.

Per-core device program, variant "direct" (default), measured-driven design:
  - loads: x host-sharded to c-major [4, 128, 8, 1024]; per batch FOUR 1 MiB
    dma_start jobs ([128, 2, 1024] slices, 8 KiB per-partition descriptors).
    Measured: chunked jobs sustain ~1.1 TB/s vs ~0.6 TB/s for one 4 MiB job,
    so the 16.8 MB read floor is ~15.5 us.
  - DVE: the serial LIF recurrence, one fused custom-DVE op per step
        u_t = select(u_{t-1} < 0.5, TAU*u_{t-1}, 0) + x_t
    (bitwise-exact vs the jax reference) plus the first LIF_G8 spike steps
    per batch (tensor_scalar is_ge -> uint8 {0,1}).
  - ACT: remaining spike steps via Sign(u - 0.5) -> uint8 {255,0,1}, two
    steps fused per instruction where possible (the paired membrane tiles
    are column-adjacent so one Sign covers [128, 2048]).
  - spikes accumulate per batch in a [128, 8*1024] uint8 tile; ONE 1 MiB
    store per batch issued from the otherwise-idle Pool (SWDGE) ring so
    store jobs never queue behind load jobs on SP.
  - host maps uint8 (y == 1) -> fp32: is_ge gives {0,1}, Sign gives
    {255,0,1}; spike == 1 under both, so no correction pass is needed.
  - measured engine costs: DVE LIF op ~550 ns, ACT Sign ~0.8/1.5 us
    (1024/2048 wide); both engines land ~18-20 us busy, just above the
    ~15.5 us DMA floor.
All arithmetic is fp32 and rounds identically to the jax reference, so the
spike train matches bitwise.

Variant "fused" is the previous baseline (all spikes on ACT, one 4 MiB
store at the end, unchunked loads).
"""

import os
import numpy as np

B, T, C, H, W = 32, 8, 128, 32, 32
HW = H * W
N_CORES = 8
BPC = B // N_CORES  # batches per core
TAU = 0.25
THRESH = 0.5

_nc_cache = {}
LAST_RESULTS = None
_LIF_OP = None


def _register_lif_op():
    """Register the fused LIF-step custom DVE op with concourse's runtime
    table (the documented extension point is appending to dve_ops.OPS).

        out = select(in0 < s0, in0 * s1, 0) + in1
            = TAU*u*[u < THRESH] + x      (s0=THRESH, s1=TAU)

    One DVE instruction per time step instead of two scalar_tensor_tensor
    passes; exact fp32 (mult by 2^-2 exact, one rounding add)."""
    global _LIF_OP
    if _LIF_OP is not None:
        return _LIF_OP
    from concourse import dve_ops
    from concourse.dve_spec import (
        Spec,
        Src0,
        Src1,
        C0,
        C1,
        Zero,
        select,
        lower,
        _has_src1,
    )
    from concourse.dve_uop import DveOpSpec

    name = "LIF_STEP_ANT"
    for op in dve_ops.OPS:
        if op.name == name:
            _LIF_OP = op
            return op

    body = select(Src0 < C0, Src0 * C1, Zero) + Src1

    def _ref(in0, in1, s0, s1, imm2):
        return (
            np.where(in0 < s0, in0.astype(np.float32) * s1, 0.0).astype(np.float32)
            + in1
        )

    spec = Spec(body=body, reference=_ref)
    opcode = dve_ops._CUSTOM_DVE_ROW_BASE + len(dve_ops.OPS)
    assert opcode < 0x20
    shas = {}
    for ver in ("v3", "v4"):
        try:
            uops = lower(spec, ver=ver)
        except Exception:
            continue
        shas[ver] = DveOpSpec(
            name=name, opcode=opcode, uops=uops, rd1_en=_has_src1(spec)
        ).sha(ver)
    op = dve_ops.DveOp(name, spec, subdim=False, uops_sha=shas)
    dve_ops.OPS.append(op)
    dve_ops._SUB_OPCODE_FOR_NAME[name] = opcode
    dve_ops.CUSTOM_DVE_SPECS[name] = spec
    _LIF_OP = op
    return op


def build_bass(reps=1, variant="direct"):
    """Per-core Bass program. reps>1 repeats the whole computation for
    loop-delta hardware timing (outputs are rewritten identically)."""
    import concourse.bacc as bacc
    import concourse.mybir as mybir
    from concourse.tile import TileContext

    f32 = mybir.dt.float32
    u8 = mybir.dt.uint8
    Alu = mybir.AluOpType
    AF = mybir.ActivationFunctionType
    lif_op = _register_lif_op()

    nc = bacc.Bacc("TRN2", target_bir_lowering=False)
    x = nc.dram_tensor("x", [BPC, C, T, HW], f32, kind="ExternalInput")
    y = nc.dram_tensor("y", [BPC, C, T, HW], u8, kind="ExternalOutput")

    if variant == "fused":
        with TileContext(nc) as tc:
            with (
                tc.tile_pool(name="xp", bufs=3) as xp,
                tc.tile_pool(name="up", bufs=3) as up,
                tc.tile_pool(name="yp", bufs=2) as yp,
                tc.tile_pool(name="cp", bufs=1) as cp,
            ):
                neg_thresh = cp.tile([C, 1], f32)
                nc.vector.memset(neg_thresh[:], -THRESH)
                for _rep in range(reps):
                    for b in range(BPC):
                        xb = xp.tile([C, T, HW], f32, tag="xb")
                        nc.sync.dma_start(xb[:], x[b])
                        if b == 0:
                            yg = yp.tile([C, BPC, T, HW], u8, tag="yb")
                        yb = yg[:, b]
                        u = None
                        for t in range(T):
                            xt = xb[:, t, :]
                            if t == 0:
                                u = xt
                            else:
                                un = up.tile([C, HW], f32, tag="u")
                                nc.vector._custom_dve(
                                    lif_op,
                                    out=un[:],
                                    in0=u if t == 1 else u[:],
                                    in1=xt,
                                    s0=THRESH,
                                    s1=TAU,
                                )
                                u = un
                            uin = u if t == 0 else u[:]
                            nc.scalar.activation(
                                yb[:, t, :], uin, AF.Sign, bias=neg_thresh[:]
                            )
                        if b == BPC - 1:
                            nc.scalar.dma_start(
                                y[:].rearrange("b c t w -> c b t w"), yg[:]
                            )
        nc.compile()
        return nc

    assert variant == "direct"
    g8 = int(os.environ.get("LIF_G8", "2"))  # spike steps on DVE per batch
    # ACT handles steps g8..7, two per Sign instruction where possible.
    # t=0's membrane is x_0 inside the load tile, so it can never pair.
    act_steps = list(range(g8, T))
    pairs = []
    i = 0
    while i < len(act_steps):
        n = 2 if i + 1 < len(act_steps) and act_steps[i] != 0 else 1
        pairs.append(tuple(act_steps[i : i + n]))
        i += n
    pair_of = {t: p for p in pairs for t in p}

    with TileContext(nc) as tc:
        with (
            tc.tile_pool(name="xp", bufs=3) as xp,
            tc.tile_pool(name="up", bufs=3) as up,
            tc.tile_pool(name="yp", bufs=2) as yp,
            tc.tile_pool(name="cp", bufs=1) as cp,
        ):
            neg_thresh = cp.tile([C, 1], f32)
            nc.vector.memset(neg_thresh[:], -THRESH)
            for _rep in range(reps):
                for b in range(BPC):
                    xb = xp.tile([C, T, HW], f32, tag="xb")
                    for h in range(0, T, 2):
                        nc.sync.dma_start(xb[:, h : h + 2], x[b, :, h : h + 2])
                    sy = yp.tile([C, T, HW], u8, tag="sy")
                    uap = {0: xb[:, 0, :]}
                    ptile = {}
                    for t in range(T):
                        if t >= 1:
                            p = pair_of.get(t)
                            if p is not None and len(p) == 2:
                                if t == p[0]:
                                    ptile[p] = up.tile([C, 2, HW], f32, tag="u2", name="u2")
                                dst = ptile[p][:, t - p[0], :]
                            else:
                                u1 = up.tile([C, HW], f32, tag="u1", name="u1")
                                dst = u1[:]
                            nc.vector._custom_dve(
                                lif_op,
                                out=dst,
                                in0=uap[t - 1],
                                in1=xb[:, t, :],
                                s0=THRESH,
                                s1=TAU,
                            )
                            uap[t] = dst
                        if t < g8:
                            # DVE spike: {0,1} uint8
                            nc.vector.tensor_scalar(
                                sy[:, t, :], uap[t], THRESH, None, Alu.is_ge
                            )
                        else:
                            p = pair_of[t]
                            if t == p[-1]:
                                # ACT spike(s): Sign -> {255,0,1} uint8
                                if len(p) == 2:
                                    nc.scalar.activation(
                                        sy[:, p[0] : p[0] + 2, :],
                                        ptile[p][:],
                                        AF.Sign,
                                        bias=neg_thresh[:],
                                    )
                                else:
                                    nc.scalar.activation(
                                        sy[:, t, :],
                                        uap[t],
                                        AF.Sign,
                                        bias=neg_thresh[:],
                                    )
                    # store off the SP ring so store jobs never queue
                    # behind load jobs
                    store_eng = {
                        "pool": nc.gpsimd,
                        "act": nc.scalar,
                        "sp": nc.sync,
                        "dve": nc.vector,
                    }[os.environ.get("LIF_STORE", "act")]
                    if os.environ.get("LIF_NOSTORE", "0") == "1":
                        # diagnostic: only the last rep stores (timing-only)
                        if _rep == reps - 1:
                            store_eng.dma_start(y[b], sy[:])
                    else:
                        store_eng.dma_start(y[b], sy[:])
    nc.compile()
    return nc


def _get_nc():
    variant = os.environ.get("LIF_VARIANT", "direct")
    key = (variant, os.environ.get("LIF_G8", "2"), os.environ.get("LIF_STORE", "act"))
    if key not in _nc_cache:
        _nc_cache[key] = build_bass(variant=variant)
    return _nc_cache[key], variant


def kernel(x):
    global LAST_RESULTS
    from concourse import bass_utils

    x = np.asarray(x)
    assert x.shape == (B, T, C, H, W) and x.dtype == np.float32
    # shard to per-core c-major [BPC, C, T, HW] (the copy happens anyway)
    xs = np.moveaxis(x.reshape(B, T, C, HW), 1, 2)
    nc, variant = _get_nc()
    in_maps = [
        {"x": np.ascontiguousarray(xs[i * BPC : (i + 1) * BPC])}
        for i in range(N_CORES)
    ]
    res = bass_utils.run_bass_kernel_spmd(
        nc,
        in_maps,
        core_ids=list(range(N_CORES)),
        trace=bool(int(os.environ.get("LIF_TRACE", "0"))),
    )
    LAST_RESULTS = res
    out = np.empty((B, T, C, HW), dtype=np.float32)
    for i in range(N_CORES):
        # DVE is_ge gives {0,1}; ACT Sign gives {-1,0,+1} = {255,0,1} in
        # uint8. spike == 1 under both conventions.
        yi = np.moveaxis(res.results[i]["y"], 1, 2)
        out[i * BPC : (i + 1) * BPC] = yi == 1
    return out.reshape(B, T, C, H, W)


# revision 14
# speedup vs baseline: 26.8813x; 1.4884x over previous
"""LIF spike kernel for Trainium2 (Bass/Tile), data-parallel over 8 NeuronCores.

Problem: x [32, 8, 128, 32, 32] fp32 -> spikes [32, 8, 128, 32, 32] fp32
    mem_t = mem_{t-1} * 0.25 + x_t ; spike = (mem >= 0.5) ; mem *= (1 - spike)

Sharding: batch dim (32) split 4-per-core across 8 cores; no cross-core comm.

Per-core device program, variant "direct" (default), measured-driven design:
  - loads: x host-sharded to c-major [4, 128, 8, 1024]; per batch FOUR 1 MiB
    dma_start jobs ([128, 2, 1024] slices, 8 KiB per-partition descriptors).
    Measured: chunked jobs sustain ~1.1 TB/s vs ~0.6 TB/s for one 4 MiB job,
    so the 16.8 MB read floor is ~15.5 us.
  - DVE: the serial LIF recurrence, one fused custom-DVE op per step
        u_t = select(u_{t-1} < 0.5, TAU*u_{t-1}, 0) + x_t
    (bitwise-exact vs the jax reference) plus the first LIF_G8 spike steps
    per batch (tensor_scalar is_ge -> uint8 {0,1}).
  - ACT: remaining spike steps via Sign(u - 0.5) -> uint8 {255,0,1}, two
    steps fused per instruction where possible (the paired membrane tiles
    are column-adjacent so one Sign covers [128, 2048]).
  - spikes accumulate per batch in a [128, 8*1024] uint8 tile; ONE 1 MiB
    store per batch issued from the otherwise-idle Pool (SWDGE) ring so
    store jobs never queue behind load jobs on SP.
  - host maps uint8 (y == 1) -> fp32: is_ge gives {0,1}, Sign gives
    {255,0,1}; spike == 1 under both, so no correction pass is needed.
  - measured engine costs: DVE LIF op ~550 ns, ACT Sign ~0.8/1.5 us
    (1024/2048 wide); both engines land ~18-20 us busy, just above the
    ~15.5 us DMA floor.
All arithmetic is fp32 and rounds identically to the jax reference, so the
spike train matches bitwise.

Variant "fused" is the previous baseline (all spikes on ACT, one 4 MiB
store at the end, unchunked loads).
"""

import os
import numpy as np

B, T, C, H, W = 32, 8, 128, 32, 32
HW = H * W
N_CORES = 8
BPC = B // N_CORES  # batches per core
TAU = 0.25
THRESH = 0.5

_nc_cache = {}
LAST_RESULTS = None
_LIF_OP = None


def _register_lif_op():
    """Register the fused LIF-step custom DVE op with concourse's runtime
    table (the documented extension point is appending to dve_ops.OPS).

        out = select(in0 < s0, in0 * s1, 0) + in1
            = TAU*u*[u < THRESH] + x      (s0=THRESH, s1=TAU)

    One DVE instruction per time step instead of two scalar_tensor_tensor
    passes; exact fp32 (mult by 2^-2 exact, one rounding add)."""
    global _LIF_OP
    if _LIF_OP is not None:
        return _LIF_OP
    from concourse import dve_ops
    from concourse.dve_spec import (
        Spec,
        Src0,
        Src1,
        C0,
        C1,
        Zero,
        select,
        lower,
        _has_src1,
    )
    from concourse.dve_uop import DveOpSpec

    name = "LIF_STEP_ANT"
    for op in dve_ops.OPS:
        if op.name == name:
            _LIF_OP = op
            return op

    body = select(Src0 < C0, Src0 * C1, Zero) + Src1

    def _ref(in0, in1, s0, s1, imm2):
        return (
            np.where(in0 < s0, in0.astype(np.float32) * s1, 0.0).astype(np.float32)
            + in1
        )

    spec = Spec(body=body, reference=_ref)
    opcode = dve_ops._CUSTOM_DVE_ROW_BASE + len(dve_ops.OPS)
    assert opcode < 0x20
    shas = {}
    for ver in ("v3", "v4"):
        try:
            uops = lower(spec, ver=ver)
        except Exception:
            continue
        shas[ver] = DveOpSpec(
            name=name, opcode=opcode, uops=uops, rd1_en=_has_src1(spec)
        ).sha(ver)
    op = dve_ops.DveOp(name, spec, subdim=False, uops_sha=shas)
    dve_ops.OPS.append(op)
    dve_ops._SUB_OPCODE_FOR_NAME[name] = opcode
    dve_ops.CUSTOM_DVE_SPECS[name] = spec
    _LIF_OP = op
    return op


def build_bass(reps=1, variant="direct"):
    """Per-core Bass program. reps>1 repeats the whole computation for
    loop-delta hardware timing (outputs are rewritten identically)."""
    import concourse.bacc as bacc
    import concourse.mybir as mybir
    from concourse.tile import TileContext

    f32 = mybir.dt.float32
    u8 = mybir.dt.uint8
    Alu = mybir.AluOpType
    AF = mybir.ActivationFunctionType
    lif_op = _register_lif_op()

    nc = bacc.Bacc("TRN2", target_bir_lowering=False)
    x = nc.dram_tensor("x", [BPC, C, T, HW], f32, kind="ExternalInput")
    y = nc.dram_tensor("y", [BPC, C, T, HW], u8, kind="ExternalOutput")

    if variant == "fused":
        with TileContext(nc) as tc:
            with (
                tc.tile_pool(name="xp", bufs=3) as xp,
                tc.tile_pool(name="up", bufs=3) as up,
                tc.tile_pool(name="yp", bufs=2) as yp,
                tc.tile_pool(name="cp", bufs=1) as cp,
            ):
                neg_thresh = cp.tile([C, 1], f32)
                nc.vector.memset(neg_thresh[:], -THRESH)
                for _rep in range(reps):
                    for b in range(BPC):
                        xb = xp.tile([C, T, HW], f32, tag="xb")
                        nc.sync.dma_start(xb[:], x[b])
                        if b == 0:
                            yg = yp.tile([C, BPC, T, HW], u8, tag="yb")
                        yb = yg[:, b]
                        u = None
                        for t in range(T):
                            xt = xb[:, t, :]
                            if t == 0:
                                u = xt
                            else:
                                un = up.tile([C, HW], f32, tag="u")
                                nc.vector._custom_dve(
                                    lif_op,
                                    out=un[:],
                                    in0=u if t == 1 else u[:],
                                    in1=xt,
                                    s0=THRESH,
                                    s1=TAU,
                                )
                                u = un
                            uin = u if t == 0 else u[:]
                            nc.scalar.activation(
                                yb[:, t, :], uin, AF.Sign, bias=neg_thresh[:]
                            )
                        if b == BPC - 1:
                            nc.scalar.dma_start(
                                y[:].rearrange("b c t w -> c b t w"), yg[:]
                            )
        nc.compile()
        return nc

    assert variant == "direct"
    g8 = int(os.environ.get("LIF_G8", "2"))  # spike steps on DVE per batch
    # ACT handles steps g8..7, two per Sign instruction where possible.
    # t=0's membrane is x_0 inside the load tile, so it can never pair.
    act_steps = list(range(g8, T))
    pairs = []
    i = 0
    while i < len(act_steps):
        n = 2 if i + 1 < len(act_steps) and act_steps[i] != 0 else 1
        pairs.append(tuple(act_steps[i : i + n]))
        i += n
    pair_of = {t: p for p in pairs for t in p}

    with TileContext(nc) as tc:
        with (
            tc.tile_pool(name="xp", bufs=3) as xp,
            tc.tile_pool(name="up", bufs=3) as up,
            tc.tile_pool(name="yp", bufs=2) as yp,
            tc.tile_pool(name="cp", bufs=1) as cp,
        ):
            neg_thresh = cp.tile([C, 1], f32)
            nc.vector.memset(neg_thresh[:], -THRESH)
            store_mode = os.environ.get("LIF_SMODE", "batch")
            for _rep in range(reps):
                yg = None
                for b in range(BPC):
                    xb = xp.tile([C, T, HW], f32, tag="xb")
                    for h in range(0, T, 2):
                        nc.sync.dma_start(xb[:, h : h + 2], x[b, :, h : h + 2])
                    if store_mode == "rep":
                        if b == 0:
                            yg = yp.tile([C, BPC, T, HW], u8, tag="yg")
                        sy = yg[:, b]
                    else:
                        sy = yp.tile([C, T, HW], u8, tag="sy")
                    uap = {0: xb[:, 0, :]}
                    ptile = {}
                    for t in range(T):
                        if t >= 1:
                            p = pair_of.get(t)
                            if p is not None and len(p) == 2:
                                if t == p[0]:
                                    ptile[p] = up.tile([C, 2, HW], f32, tag="u2", name="u2")
                                dst = ptile[p][:, t - p[0], :]
                            else:
                                u1 = up.tile([C, HW], f32, tag="u1", name="u1")
                                dst = u1[:]
                            nc.vector._custom_dve(
                                lif_op,
                                out=dst,
                                in0=uap[t - 1],
                                in1=xb[:, t, :],
                                s0=THRESH,
                                s1=TAU,
                            )
                            uap[t] = dst
                        if t < g8:
                            # DVE spike: {0,1} uint8
                            nc.vector.tensor_scalar(
                                sy[:, t, :], uap[t], THRESH, None, Alu.is_ge
                            )
                        else:
                            p = pair_of[t]
                            if t == p[-1]:
                                # ACT spike(s): Sign -> {255,0,1} uint8
                                if len(p) == 2:
                                    nc.scalar.activation(
                                        sy[:, p[0] : p[0] + 2, :],
                                        ptile[p][:],
                                        AF.Sign,
                                        bias=neg_thresh[:],
                                    )
                                else:
                                    nc.scalar.activation(
                                        sy[:, t, :],
                                        uap[t],
                                        AF.Sign,
                                        bias=neg_thresh[:],
                                    )
                    # store off the SP ring so store jobs never queue
                    # behind load jobs
                    store_eng = {
                        "pool": nc.gpsimd,
                        "act": nc.scalar,
                        "sp": nc.sync,
                        "dve": nc.vector,
                    }[os.environ.get("LIF_STORE", "act")]
                    nostore = (
                        os.environ.get("LIF_NOSTORE", "0") == "1"
                        and _rep != reps - 1
                    )
                    if store_mode == "rep":
                        if b == BPC - 1 and not nostore:
                            store_eng.dma_start(
                                y[:].rearrange("b c t w -> c b t w"), yg[:]
                            )
                    elif not nostore:
                        store_eng.dma_start(y[b], sy[:])
    nc.compile()
    return nc


def _get_nc():
    variant = os.environ.get("LIF_VARIANT", "direct")
    key = (variant, os.environ.get("LIF_G8", "2"), os.environ.get("LIF_STORE", "act"), os.environ.get("LIF_SMODE", "batch"))
    if key not in _nc_cache:
        _nc_cache[key] = build_bass(variant=variant)
    return _nc_cache[key], variant


def kernel(x):
    global LAST_RESULTS
    from concourse import bass_utils

    x = np.asarray(x)
    assert x.shape == (B, T, C, H, W) and x.dtype == np.float32
    # shard to per-core c-major [BPC, C, T, HW] (the copy happens anyway)
    xs = np.moveaxis(x.reshape(B, T, C, HW), 1, 2)
    nc, variant = _get_nc()
    in_maps = [
        {"x": np.ascontiguousarray(xs[i * BPC : (i + 1) * BPC])}
        for i in range(N_CORES)
    ]
    res = bass_utils.run_bass_kernel_spmd(
        nc,
        in_maps,
        core_ids=list(range(N_CORES)),
        trace=bool(int(os.environ.get("LIF_TRACE", "0"))),
    )
    LAST_RESULTS = res
    out = np.empty((B, T, C, HW), dtype=np.float32)
    for i in range(N_CORES):
        # DVE is_ge gives {0,1}; ACT Sign gives {-1,0,+1} = {255,0,1} in
        # uint8. spike == 1 under both conventions.
        yi = np.moveaxis(res.results[i]["y"], 1, 2)
        out[i * BPC : (i + 1) * BPC] = yi == 1
    return out.reshape(B, T, C, H, W)
